# revision 1
# baseline (speedup 1.0000x reference)
"""Binned spectra (per-row histogram) Trainium2 kernel.

Algorithm (per batch row, 9900 = 100*99 bins):
  bin = trunc((mz-10)/0.1)  in [0, 9900) for valid peaks
  hi = bin // 99  in [0, 100);  lo = bin - 99*hi  in [0, 99)
  hist[hi, lo] = sum_p val_p * d(hi_p==hi) * d(lo_p==lo)   (outer product, PE matmul)
  where val_p = sqrt(intensity_p) * (10 <= mz_p < 1000)

The PE computes hist = A^T @ C with A[p,hi] = d(hi_p==hi) (bf16 one-hot,
stationary) and C[p,lo] = d(lo_p==lo)*val_p.  For fp32 accuracy val is split
val = bf16(val) + bf16(val - bf16(val)); both bf16 pieces are streamed as
separate moving operands accumulating into the same fp32 PSUM tile, so the
result carries ~16 mantissa bits per value (measured rel err ~2e-6 vs the
fp32 reference).  The fp32 division (mz-10)/0.1f is emulated exactly
(Dekker residual + 1-eps correction) because the DVE has no divide ALU op,
and floor() is computed robustly to the hardware's f32->i32 rounding mode.

One-hot mask builds dominate; they are round-robined across DVE (~54%),
Pool/gpsimd (~28%) and ACT (~25% of the hi-masks, via |x| then relu(1-x)).

Data parallel over 8 NeuronCores: each core takes 512 of the 4096 rows.
"""

import sys

sys.path.insert(0, "/opt/trn_rl_repo")

import numpy as np

import concourse.bass as bass
import concourse.tile as tile
from concourse import bacc, mybir
from concourse.bass_utils import run_bass_kernel_spmd
from concourse.masks import make_identity

N_CORES = 8
B, P = 4096, 1024
NUM_BINS = 9900
H, L = 100, 99
RT = 128  # rows per row-tile (SBUF partition dim)
NCHUNK = P // 128  # peak chunks per row

f32 = mybir.dt.float32
bf16 = mybir.dt.bfloat16
i32 = mybir.dt.int32

INV99 = float(np.float32(1.0) / np.float32(99.0))
# u / 0.1f == u * 10 * (1 - EPS_D) exactly:  10*0.1f = 1 + 1.49e-8
EPS_D = float(1.0 - 1.0 / (10.0 * np.float64(np.float32(0.1))))


def build_program(
    rows_per_core: int,
    exact: bool = True,
    # Real-HW measured (chained-execution bench): gpsimd tensor_scalar is
    # ~10x the cost model, so Pool gets NO mask work; DVE ~60% / ACT ~40%.
    pool_pat: tuple = (1, ()),
    mask_bufs: int = 24,
    mm_bufs: int = 3,
    act_pat: tuple = (5, (0, 2)),
    act_c1: bool = False,
    stage_pat: tuple = (1, (0,)),
    stage_rows: int = 2,
    fuse_c: bool = False,
    scr_bufs: int = 1,
):
    """Build the (single-core SPMD) Bass program for rows_per_core rows.

    exact=True: values enter the bf16 matmul split in two bf16 pieces
    (fp32-accurate result, ~rel 3e-6).  exact=False: single bf16 piece
    (~rel 5e-4) but ~30% fewer mask ops.
    """
    from contextlib import ExitStack

    assert rows_per_core % RT == 0
    nt = rows_per_core // RT
    cw = 2 * H if exact else H  # moving-operand width per chunk

    nc = bacc.Bacc(
        "TRN2", target_bir_lowering=False, debug=False, num_devices=N_CORES
    )
    mz_d = nc.dram_tensor("mz", [rows_per_core, P], f32, kind="ExternalInput").ap()
    it_d = nc.dram_tensor(
        "intensities", [rows_per_core, P], f32, kind="ExternalInput"
    ).ap()
    out_d = nc.dram_tensor(
        "out", [rows_per_core, NUM_BINS], f32, kind="ExternalOutput"
    ).ap()

    with tile.TileContext(nc) as tc, ExitStack() as ctx:
        cpool = ctx.enter_context(tc.tile_pool(name="consts", bufs=1))
        inpool = ctx.enter_context(tc.tile_pool(name="inp", bufs=2))
        tpsum = ctx.enter_context(tc.tile_pool(name="tpsum", bufs=2, space="PSUM"))
        scr = ctx.enter_context(tc.tile_pool(name="scratch", bufs=scr_bufs))
        wide = ctx.enter_context(tc.tile_pool(name="wide", bufs=2))
        maskp = ctx.enter_context(tc.tile_pool(name="masks", bufs=mask_bufs))
        mmpsum = ctx.enter_context(
            tc.tile_pool(name="mmpsum", bufs=mm_bufs, space="PSUM")
        )
        histp = ctx.enter_context(tc.tile_pool(name="hist", bufs=1))

        # constants
        ident = cpool.tile([128, 128], f32, tag="ident")
        make_identity(nc, ident[:])
        iota_i = cpool.tile([128, H], i32, tag="iota_i")
        nc.gpsimd.iota(iota_i[:], pattern=[[1, H]], base=0, channel_multiplier=0)
        iota_bf = cpool.tile([128, H], bf16, tag="iota_bf")
        nc.vector.tensor_copy(iota_bf[:], iota_i[:])
        if fuse_c:
            # (0,0,1,1,...,99,99) for the interleaved fused C build
            iota2_i = cpool.tile([128, 2 * H], i32, tag="iota2_i")
            nc.gpsimd.iota(
                iota2_i[:], pattern=[[1, H], [0, 2]], base=0, channel_multiplier=0
            )
            iota2_bf = cpool.tile([128, 2 * H], bf16, tag="iota2_bf")
            nc.vector.tensor_copy(iota2_bf[:], iota2_i[:])

        out_v = out_d.rearrange("(t r) (h l) -> t h r l", r=RT, l=L)

        for t in range(nt):
            rs = t * RT
            mzt = inpool.tile([128, P], f32, tag="mz")
            nc.sync.dma_start(mzt[:], mz_d[rs : rs + RT, :])
            itt = inpool.tile([128, P], f32, tag="it")
            nc.sync.dma_start(itt[:], it_d[rs : rs + RT, :])

            # ---- transpose to peak-major + per-chunk math ----
            uT = scr.tile([128, P], f32, tag="uT")  # mz-10, peak-major
            sqT = scr.tile([128, P], f32, tag="sqT")  # sqrt(intensity)
            for c in range(NCHUNK):
                cs = slice(c * 128, (c + 1) * 128)
                pz = tpsum.tile([128, 128], f32, tag="tp")
                nc.tensor.transpose(pz[:], mzt[:, cs], ident[:])
                nc.vector.tensor_scalar(
                    uT[:, cs], pz[:], 10.0, None, mybir.AluOpType.subtract
                )
                pz2 = tpsum.tile([128, 128], f32, tag="tp")
                nc.tensor.transpose(pz2[:], itt[:, cs], ident[:])
                nc.scalar.sqrt(sqT[:, cs], pz2[:])

            # ---- wide math (all peak-major [128, 1024]) ----
            # q = u / 0.1 (exactly-rounded fp32 division, matches reference).
            # The DVE has no divide ALU op; emulate:  u/0.1f = u*10*(1-eps)
            # with q_hi = RN(8u+2u) (8u, 2u exact), Dekker residual
            # e = 2u - (q_hi - 8u), correction c2 = e - eps*q_hi, and
            # q = RN(q_hi + c2) == RN(u/0.1f).
            b2 = scr.tile([128, P], f32, tag="b2")
            nc.vector.tensor_scalar(b2[:], uT[:], 2.0, None, mybir.AluOpType.mult)
            qhi = scr.tile([128, P], f32, tag="qhi")
            nc.vector.scalar_tensor_tensor(
                qhi[:],
                in0=uT[:],
                scalar=8.0,
                in1=b2[:],
                op0=mybir.AluOpType.mult,
                op1=mybir.AluOpType.add,
            )
            bv = scr.tile([128, P], f32, tag="bv")
            nc.vector.scalar_tensor_tensor(
                bv[:],
                in0=uT[:],
                scalar=-8.0,
                in1=qhi[:],
                op0=mybir.AluOpType.mult,
                op1=mybir.AluOpType.add,
            )
            ederr = scr.tile([128, P], f32, tag="a8")
            nc.vector.tensor_tensor(ederr[:], b2[:], bv[:], mybir.AluOpType.subtract)
            c2t = scr.tile([128, P], f32, tag="bv")
            nc.vector.scalar_tensor_tensor(
                c2t[:],
                in0=qhi[:],
                scalar=-EPS_D,
                in1=ederr[:],
                op0=mybir.AluOpType.mult,
                op1=mybir.AluOpType.add,
            )
            qT = scr.tile([128, P], f32, tag="qT")
            nc.vector.tensor_tensor(qT[:], qhi[:], c2t[:], mybir.AluOpType.add)
            # binf = floor(q) robust to convert rounding mode
            itmp = scr.tile([128, P], i32, tag="itmp")
            nc.vector.tensor_copy(itmp[:], qT[:])
            ftmp = scr.tile([128, P], f32, tag="ftmp")
            nc.scalar.copy(ftmp[:], itmp[:])
            cond = scr.tile([128, P], f32, tag="cond")
            nc.vector.tensor_tensor(cond[:], ftmp[:], qT[:], mybir.AluOpType.is_gt)
            binT = scr.tile([128, P], f32, tag="binT")
            nc.vector.tensor_tensor(binT[:], ftmp[:], cond[:], mybir.AluOpType.subtract)
            # hi = floor((bin + 0.5) / 99)
            hq = scr.tile([128, P], f32, tag="qT")
            nc.vector.tensor_scalar(
                hq[:], binT[:], 0.5, INV99, mybir.AluOpType.add, mybir.AluOpType.mult
            )
            itmp2 = scr.tile([128, P], i32, tag="itmp")
            nc.vector.tensor_copy(itmp2[:], hq[:])
            ftmp2 = scr.tile([128, P], f32, tag="ftmp")
            nc.scalar.copy(ftmp2[:], itmp2[:])
            cond2 = scr.tile([128, P], f32, tag="cond")
            nc.vector.tensor_tensor(cond2[:], ftmp2[:], hq[:], mybir.AluOpType.is_gt)
            hiT = wide.tile([128, P], f32, tag="hiT")
            nc.vector.tensor_tensor(
                hiT[:], ftmp2[:], cond2[:], mybir.AluOpType.subtract
            )
            # lo = bin - 99*hi
            loT = wide.tile([128, P], f32, tag="loT")
            nc.vector.scalar_tensor_tensor(
                loT[:],
                in0=hiT[:],
                scalar=-99.0,
                in1=binT[:],
                op0=mybir.AluOpType.mult,
                op1=mybir.AluOpType.add,
            )
            # val = sqrt(it) * (u >= 0) * (u < 990)
            v1 = scr.tile([128, P], f32, tag="v1")
            nc.vector.scalar_tensor_tensor(
                v1[:],
                in0=uT[:],
                scalar=0.0,
                in1=sqT[:],
                op0=mybir.AluOpType.is_ge,
                op1=mybir.AluOpType.mult,
            )
            vT = scr.tile([128, P], f32, tag="vT")
            nc.vector.scalar_tensor_tensor(
                vT[:],
                in0=uT[:],
                scalar=990.0,
                in1=v1[:],
                op0=mybir.AluOpType.is_lt,
                op1=mybir.AluOpType.mult,
            )
            # val splitting for fp32 accuracy: vhif = f32(bf16(v)) is exactly
            # bf16-representable, so mask ops' bf16 output conversion is exact
            # on any engine; vlo = v - vhif carries the residual.
            vhib = scr.tile([128, P], bf16, tag="vhib")
            nc.scalar.copy(vhib[:], vT[:])
            vhif = wide.tile([128, P], f32, tag="vhif")
            nc.scalar.copy(vhif[:], vhib[:])
            vloT = wide.tile([128, P], f32, tag="vloT")
            nc.vector.tensor_tensor(
                vloT[:], vT[:], vhif[:], mybir.AluOpType.subtract
            )
            if act_c1:
                nvhi = wide.tile([128, P], f32, tag="nvhi")
                nc.vector.tensor_scalar(
                    nvhi[:], vhif[:], -1.0, None, mybir.AluOpType.mult
                )
            if fuse_c:
                # interleaved (vhi, vlo) pairs per peak, bf16
                v2 = wide.tile([128, 2 * P], bf16, tag="v2")
                v2v = v2[:].rearrange("p (c two) -> p c two", two=2)
                nc.scalar.copy(v2v[:, :, 0], vhif[:])
                nc.scalar.copy(v2v[:, :, 1], vloT[:])

            # ---- per-row one-hot matmuls ----
            # Mask builds are the bottleneck: split them DVE / Pool (gpsimd).
            # Both val-pieces accumulate into the same PSUM tile (hardware
            # allows only one PSUM operand per vector op, so no psum+psum add).
            hist = histp.tile([100, RT * H], f32, tag="hist")
            pr_full = None
            for r in range(RT):
                # stage_rows consecutive rows share one PSUM tile (one bank
                # per row, 512-f32 pitch) so staging copies can batch.
                if r % stage_rows == 0:
                    pr_full = mmpsum.tile([100, stage_rows * 512], f32, tag="mm")
                roff = (r % stage_rows) * 512
                pr = pr_full[:, roff : roff + H]
                for c in range(NCHUNK):
                    col = c * 128 + r
                    idx = r * NCHUNK + c
                    # round-robin engines: Pool takes a fraction of mask ops
                    eng = (
                        nc.gpsimd
                        if idx % pool_pat[0] in pool_pat[1]
                        else nc.vector
                    )
                    a = maskp.tile([128, H], bf16, tag="A")
                    if act_pat and idx % act_pat[0] in act_pat[1]:
                        # build the hi one-hot on the ACT engine:
                        # tmp = |hi - iota|; a = relu(1 - tmp)
                        at = maskp.tile([128, H], bf16, tag="At")
                        nc.scalar.activation(
                            at[:],
                            iota_bf[:],
                            mybir.ActivationFunctionType.Abs,
                            bias=hiT[:, col : col + 1],
                            scale=-1.0,
                        )
                        nc.scalar.activation(
                            a[:],
                            at[:],
                            mybir.ActivationFunctionType.Relu,
                            bias=1.0,
                            scale=-1.0,
                        )
                    else:
                        eng.tensor_scalar(
                            a[:],
                            iota_bf[:],
                            hiT[:, col : col + 1],
                            None,
                            mybir.AluOpType.is_equal,
                        )
                    if fuse_c and exact:
                        # one fused op builds delta(lo)*{vhi, vlo} interleaved
                        c12i = maskp.tile([128, 2 * H], bf16, tag="C")
                        civ = c12i[:].rearrange("p (j two) -> p j two", two=2)
                        eng.scalar_tensor_tensor(
                            civ,
                            in0=iota2_bf[:].rearrange(
                                "p (j two) -> p j two", two=2
                            ),
                            scalar=loT[:, col : col + 1],
                            in1=v2[:]
                            .rearrange("p (c two) -> p c two", two=2)[
                                :, col : col + 1, :
                            ]
                            .to_broadcast([128, H, 2]),
                            op0=mybir.AluOpType.is_equal,
                            op1=mybir.AluOpType.mult,
                        )
                        nc.tensor.matmul(
                            pr[:, :],
                            lhsT=a[:],
                            rhs=civ[:, :, 0],
                            start=(c == 0),
                            stop=False,
                        )
                        nc.tensor.matmul(
                            pr[:, :],
                            lhsT=a[:],
                            rhs=civ[:, :, 1],
                            start=False,
                            stop=(c == NCHUNK - 1),
                        )
                        continue
                    c12 = maskp.tile([128, cw], bf16, tag="C")
                    if act_c1 and act_pat and idx % act_pat[0] in act_pat[1]:
                        # C1 on ACT: t = |lo - iota|; c1 = relu(vhi - vhi*t)
                        at2 = maskp.tile([128, H], bf16, tag="At2")
                        nc.scalar.activation(
                            at2[:],
                            iota_bf[:],
                            mybir.ActivationFunctionType.Abs,
                            bias=loT[:, col : col + 1],
                            scale=-1.0,
                        )
                        nc.scalar.activation(
                            c12[:, 0:H],
                            at2[:],
                            mybir.ActivationFunctionType.Relu,
                            bias=vhif[:, col : col + 1],
                            scale=nvhi[:, col : col + 1],
                        )
                    else:
                        eng.tensor_scalar(
                            c12[:, 0:H],
                            iota_bf[:],
                            loT[:, col : col + 1],
                            vhif[:, col : col + 1],
                            mybir.AluOpType.is_equal,
                            mybir.AluOpType.mult,
                        )
                    if exact:
                        eng.tensor_scalar(
                            c12[:, H : 2 * H],
                            iota_bf[:],
                            loT[:, col : col + 1],
                            vloT[:, col : col + 1],
                            mybir.AluOpType.is_equal,
                            mybir.AluOpType.mult,
                        )
                    nc.tensor.matmul(
                        pr[:, :],
                        lhsT=a[:],
                        rhs=c12[:, 0:H],
                        start=(c == 0),
                        stop=(c == NCHUNK - 1) and not exact,
                    )
                    if exact:
                        nc.tensor.matmul(
                            pr[:, :],
                            lhsT=a[:],
                            rhs=c12[:, H : 2 * H],
                            start=False,
                            stop=(c == NCHUNK - 1),
                        )
                # PSUM -> SBUF staging (batched over stage_rows), ACT/DVE split
                if (r + 1) % stage_rows == 0:
                    r0 = r + 1 - stage_rows
                    src = pr_full[:].rearrange("p (b x) -> p b x", x=512)[
                        :, :, 0:H
                    ]
                    dst = hist[:, r0 * H : (r + 1) * H]
                    if (r // stage_rows) % stage_pat[0] in stage_pat[1]:
                        nc.scalar.copy(dst, src)
                    else:
                        nc.vector.tensor_copy(dst, src)

            histv = hist[:].rearrange("h (r l) -> h r l", l=H)[:, :, 0:L]
            nc.sync.dma_start(out_v[t], histv)

    nc.compile()
    return nc


_CACHE: dict[int, object] = {}


def _get_program(rows_per_core: int):
    if rows_per_core not in _CACHE:
        _CACHE[rows_per_core] = build_program(rows_per_core)
    return _CACHE[rows_per_core]


def kernel(mz: np.ndarray, intensities: np.ndarray, trace: bool = False):
    mz = np.ascontiguousarray(np.asarray(mz, dtype=np.float32))
    intensities = np.ascontiguousarray(np.asarray(intensities, dtype=np.float32))
    bb = mz.shape[0]
    rows_per_core = bb // N_CORES
    nc = _get_program(rows_per_core)
    in_maps = []
    for i in range(N_CORES):
        sl = slice(i * rows_per_core, (i + 1) * rows_per_core)
        in_maps.append({"mz": mz[sl], "intensities": intensities[sl]})
    try:
        res = run_bass_kernel_spmd(
            nc, in_maps, core_ids=list(range(N_CORES)), trace=trace
        )
    except ModuleNotFoundError:
        # NTFF profiling hook unavailable in this environment
        res = run_bass_kernel_spmd(
            nc, in_maps, core_ids=list(range(N_CORES)), trace=False
        )
    out = np.concatenate([res.results[i]["out"] for i in range(N_CORES)], axis=0)
    if trace:
        kernel.last_exec_time_ns = res.exec_time_ns
        kernel.last_results = res
    return out


kernel.last_exec_time_ns = None

